# revision 17
# baseline (speedup 1.0000x reference)
"""Single-head causal attention on 8 TRN2 NeuronCores (Bass/Tile).

Problem: x[B=8,T=4096,C=1024] @ {Wq,Wk,Wv}[C,HS=64] -> causal softmax
attention -> out[B,T,HS].

Sharding: data-parallel over batch — core b computes batch element b with
replicated projection weights (per the sharding hint).

Per-core dataflow (matmul operands bf16, fp32 PSUM accumulation):
  - x arrives transposed AND pre-cast to bf16 on the host (xT[C,T] bf16),
    so the DMA stream is half the bytes and no on-chip cast is needed.
  - [qT;kT] = [Wq|Wk]^T @ xT per 512-wide query block (PSUM-accumulated
    over 8 c-chunks); vT likewise; vT is PE-transposed back to natural
    v[s,64] with a ones-column appended so the PV matmul also produces
    softmax row-sums for free.
  - Scores are computed transposed (weiT[s,t] = kT^T @ qT) as row-packed
    pairs: two K=64 matmuls on disjoint PE row groups run concurrently
    (kT interleaved across partition halves, qT duplicated to the upper
    half), writing the two halves of one 2-bank PSUM tile.
  - exp runs on ScalarE straight out of PSUM (one 1024-wide ACT per pair)
    with the 1/sqrt(C) scale folded in; no running-max is needed (logits
    are small).  The pair loop is software-pipelined so the PE issues the
    next pair's score matmuls before the current pair's PV — ScalarE (the
    critical engine) never waits on the PE.
  - Causality: fully-masked 128-col strips are sliced off the score
    matmuls/PV and the 128x128 diagonal strips are masked via a DVE
    multiply with a 0/1 upper-triangular tile.
  - PV accumulates outT[65,512] over s-tiles in PSUM; finalize is a PE
    transpose + DVE reciprocal of the sums column + per-row scale.
  - Walrus LDWEIGHTS double-buffering is enabled (concourse pins it off)
    so matmuls don't serialize behind their weight loads.
"""

import numpy as np

import concourse.bacc as bacc
import concourse.bass as bass
import concourse.mybir as mybir
import concourse.tile as tile
from concourse import bass_utils

B, T, C, HS = 8, 4096, 1024, 64
TB = 512                 # query-block width (PSUM bank = 512 fp32)
NJ = T // TB             # 8 query blocks
NK = C // 128            # 8 contraction chunks
NS = T // 128            # 32 key tiles
SCALE = C ** -0.5

F32 = mybir.dt.float32
BF16 = mybir.dt.bfloat16
EXP = mybir.ActivationFunctionType.Exp


def build_program():
    nc = bacc.Bacc("TRN2", target_bir_lowering=False, debug=False)

    # x arrives host-permuted to [p, j, k, t] so each query-block chunk is
    # one contiguous 8KB run per partition -> 128 fat DMA descriptors
    # instead of 1024 thin ones (descriptor-dominated otherwise)
    xT = nc.dram_tensor("xT", [128, NJ * NK * TB], BF16, kind="ExternalInput")
    wqk = nc.dram_tensor("wqk", [C, 128], BF16, kind="ExternalInput")
    wv = nc.dram_tensor("wv", [C, HS], BF16, kind="ExternalInput")
    iden = nc.dram_tensor("iden", [128, 128], F32, kind="ExternalInput")
    idenb = nc.dram_tensor("idenb", [128, 128], BF16, kind="ExternalInput")
    mask = nc.dram_tensor("mask", [128, 128], BF16, kind="ExternalInput")
    out = nc.dram_tensor("out", [T, HS], F32, kind="ExternalOutput")

    with tile.TileContext(nc) as tc:
        with (
            tc.tile_pool(name="const", bufs=1) as constp,
            tc.tile_pool(name="persist", bufs=1) as persist,
            tc.tile_pool(name="stage", bufs=2) as stg,
            tc.tile_pool(name="expp", bufs=4) as expp,
            tc.tile_pool(name="fin", bufs=8) as finp,
            tc.tile_pool(name="ps_pp", bufs=2, space=bass.MemorySpace.PSUM) as ps_pp,
            tc.tile_pool(name="ps_wei", bufs=2, space=bass.MemorySpace.PSUM) as ps_wei,
            tc.tile_pool(name="ps_out", bufs=2, space=bass.MemorySpace.PSUM) as ps_out,
        ):
            wqk_sb = constp.tile([128, NK, 128], BF16)
            wv_sb = constp.tile([128, NK, HS], BF16)
            iden_sb = constp.tile([128, 128], F32)
            idenb_sb = constp.tile([128, 128], BF16)
            mask_sb = constp.tile([128, 128], BF16)
            # constants ride the scalar HWDGE queue so the sync queue can
            # start streaming x immediately
            nc.scalar.dma_start(
                wqk_sb[:], wqk[:].rearrange("(k p) m -> p k m", p=128)
            )
            nc.scalar.dma_start(
                wv_sb[:], wv[:].rearrange("(k p) m -> p k m", p=128)
            )
            nc.scalar.dma_start(iden_sb[:], iden[:])
            nc.scalar.dma_start(idenb_sb[:], idenb[:])
            nc.scalar.dma_start(mask_sb[:], mask[:])
            # tiny dummy exp: pulls ACT_TABLE_LOAD (~2.7us) into the DMA head
            warm = finp.tile([1, 1], F32, tag="warm", bufs=1)
            nc.scalar.activation(warm[:], iden_sb[0:1, 0:1], EXP, scale=SCALE)

            # whole xT resident in SBUF (bf16, 64KB/partition), streamed in
            # per-block chunks so compute can start after the first chunk;
            # layout [p, j, k, t] keeps each chunk contiguous per partition
            xt = persist.tile([128, NJ, NK, TB], BF16)
            # keys, transposed + interleaved: pair p holds kT of s-tile 2p on
            # partitions 0-63 and of s-tile 2p+1 on partitions 64-127
            kTI = persist.tile([128, (NS // 2) * 128], BF16)
            # values + ones col, padded to 80 so each s-tile's row offset is
            # 32B-aligned (required by the xbar DMA-transpose destination)
            v_all = persist.tile([128, NS, 80], BF16)
            nc.vector.memset(v_all[:, :, HS : HS + 1], 1.0)

            # x stream: first two chunks ride the sync HWDGE ring (fast
            # first-byte — the head of the kernel waits on them), the rest
            # ride SWDGE (gpsimd) so they never contend with the small
            # latency-critical SBUF->SBUF transfers on the sync ring.
            for jj in range(NJ):
                eng = nc.sync if jj < 2 else nc.gpsimd
                eng.dma_start(
                    xt[:, jj, :, :],
                    xT[:, jj * NK * TB : (jj + 1) * NK * TB].rearrange(
                        "p (k t) -> p k t", t=TB
                    ),
                )

            # Projections run ONE BLOCK AHEAD of the pair loop so every
            # wei/PV dependency (qkt, qt2, kTI, v_all) is produced a full
            # block early — the small sync-ring DMAs then have a whole
            # block of slack and never stall the PE.
            def proj_qk(j):
                t0 = j * TB
                # [qT;kT] projection: rows 0-63 = qT, rows 64-127 = kT
                qk_ps = ps_pp.tile([128, TB], F32, tag="pp", name="qk_ps")
                for k in range(NK):
                    nc.tensor.matmul(
                        qk_ps[:], wqk_sb[:, k, :], xt[:, j, k, :],
                        start=(k == 0), stop=(k == NK - 1),
                    )
                qkt = stg.tile([128, TB], BF16, tag="qkt", name="qkt")
                nc.vector.tensor_copy(qkt[:], qk_ps[:])
                # duplicate qT onto partitions 64-127 (row-packed QK rhs)
                qt2 = stg.tile([128, TB], BF16, tag="qt2", name="qt2")
                nc.sync.dma_start(qt2[64:128, :], qkt[0:64, :])
                # interleave this block's 4 kT tiles into the pair layout:
                # even tiles -> partitions 0-63, odd tiles -> 64-127
                kt_src = qkt[64:128, :].rearrange(
                    "p (a e b) -> p a e b", e=2, b=128
                )
                kt_dst = kTI[:, 256 * j : 256 * (j + 1)].rearrange(
                    "p (a b) -> p a b", b=128
                )
                nc.sync.dma_start(kt_dst[0:64, :, :], kt_src[:, :, 0, :])
                nc.sync.dma_start(kt_dst[64:128, :, :], kt_src[:, :, 1, :])
                return qkt, qt2

            def proj_v(j):
                t0 = j * TB
                # v projection, row-packed across partition halves, then the
                # halves are folded and DMA-transposed to natural v[s,64]
                vt_ps = ps_pp.tile([128, TB], F32, tag="pp", name="vt_ps")
                for k in range(NK):
                    lo = HS * (k % 2)
                    nc.tensor.matmul(
                        vt_ps[lo : lo + HS, :], wv_sb[:, k, :],
                        xt[:, j, k, :],
                        start=(k <= 1), stop=(k >= NK - 2),
                        skip_group_check=True,
                    )
                vt_hi = stg.tile([128, TB], F32, tag="vt_hi", name="vt_hi")
                nc.vector.tensor_copy(vt_hi[64:128, :], vt_ps[64:128, :])
                vt_lo = stg.tile([HS, TB], F32, tag="vt_lo", name="vt_lo")
                nc.sync.dma_start(vt_lo[:], vt_hi[64:128, :])
                vt_sb = stg.tile([HS, TB], BF16, tag="vt_sb", name="vt_sb")
                nc.vector.tensor_add(vt_sb[:], vt_ps[0:HS, :], vt_lo[:])
                for rr in range(TB // 128):
                    nc.sync.dma_start_transpose(
                        v_all[:, 4 * j + rr, 0:HS],
                        vt_sb[:, rr * 128 : (rr + 1) * 128],
                    )

            def issue_wei(j, qkt, qt2, p):
                iA, iB = 2 * p, 2 * p + 1
                rA, rB = iA - 4 * j, iB - 4 * j
                c0A = 128 * rA if rA > 0 else 0
                c0B = 128 * rB if rB > 0 else 0
                wei = ps_wei.tile([128, 2 * TB], F32, tag="wei", name="wei")
                nc.tensor.matmul(
                    wei[:, c0A:TB],
                    kTI[0:64, 128 * p : 128 * (p + 1)],
                    qkt[0:HS, c0A:TB],
                    start=True, stop=True,
                )
                nc.tensor.matmul(
                    wei[:, TB + c0B : 2 * TB],
                    kTI[64:128, 128 * p : 128 * (p + 1)],
                    qt2[64:128, c0B:TB],
                    start=True, stop=True,
                )
                return wei, c0A, c0B, rA, rB

            cur = proj_qk(0)
            proj_v(0)
            # prime the wei pipeline (2 tiles = full psum ring)
            pend = [issue_wei(0, *cur, 0), issue_wei(0, *cur, 1)]

            for j in range(NJ):
                t0 = j * TB
                qkt, qt2 = cur
                n_pairs = 2 * j + 2

                outT_ps = ps_out.tile([HS + 1, TB], F32, tag="outT")
                for p in range(n_pairs):
                    wei, c0A, c0B, rA, rB = pend.pop(0)
                    iA, iB = 2 * p, 2 * p + 1
                    ex = expp.tile([128, 2 * TB], BF16, tag="exp")
                    # one ACT covers both halves; the dead [TB, TB+c0B) gap
                    # holds bounded stale scores and is never read by PV
                    nc.scalar.activation(
                        ex[:, c0A : 2 * TB], wei[:, c0A : 2 * TB], EXP,
                        scale=SCALE,
                    )
                    if rA >= 0:
                        nc.vector.tensor_mul(
                            ex[:, c0A : c0A + 128], ex[:, c0A : c0A + 128],
                            mask_sb[:],
                        )
                    if rB >= 0:
                        nc.vector.tensor_mul(
                            ex[:, TB + c0B : TB + c0B + 128],
                            ex[:, TB + c0B : TB + c0B + 128],
                            mask_sb[:],
                        )
                    if p + 2 < n_pairs:
                        pend.append(issue_wei(j, qkt, qt2, p + 2))
                    # next block's projections slot into the PE stream here
                    # (before PV, which can wait) so their results are ready
                    # a full block early
                    if p == 0 and j + 1 < NJ:
                        nxt = proj_qk(j + 1)
                    if p == 1 and j + 1 < NJ:
                        proj_v(j + 1)
                    nc.tensor.matmul(
                        outT_ps[:, c0A:TB],
                        v_all[:, iA, 0 : HS + 1],
                        ex[:, c0A:TB],
                        start=(p == 0), stop=False,
                        skip_group_check=True,
                    )
                    nc.tensor.matmul(
                        outT_ps[:, c0B:TB],
                        v_all[:, iB, 0 : HS + 1],
                        ex[:, TB + c0B : 2 * TB],
                        start=False, stop=(p == n_pairs - 1),
                        skip_group_check=True,
                    )

                # prime next block's first two score pairs BEFORE finalize,
                # so ScalarE's next ACT never waits behind the fin chain
                if j + 1 < NJ:
                    cur = nxt
                    pend = [
                        issue_wei(j + 1, *cur, 0),
                        issue_wei(j + 1, *cur, 1),
                    ]

                # finalize in bf16 (fp32 PE transpose runs in slow fp32
                # mode; bf16 rounding of outT adds ~0.3% well within budget)
                outT_sb = stg.tile([HS + 1, TB], BF16, tag="outT_sb")
                nc.vector.tensor_copy(outT_sb[:], outT_ps[:])
                o = finp.tile([128, TB // 128, HS], F32, tag="o", bufs=2)
                for rr in range(TB // 128):
                    fin_ps = ps_pp.tile([128, HS + 1], BF16, tag="pp")
                    nc.tensor.transpose(
                        fin_ps[:], outT_sb[:, rr * 128 : (rr + 1) * 128],
                        idenb_sb[: HS + 1, : HS + 1],
                    )
                    rec = finp.tile([128, 1], F32, tag="rec")
                    nc.vector.reciprocal(rec[:], fin_ps[:, HS : HS + 1])
                    nc.vector.tensor_scalar_mul(
                        o[:, rr, :], fin_ps[:, 0:HS], rec[:]
                    )
                # one coalesced store per block, on SWDGE (leaf transfer —
                # nothing downstream waits on it)
                nc.gpsimd.dma_start(
                    out[t0 : t0 + TB, :].rearrange("(a p) h -> p a h", p=128),
                    o[:],
                )

    nc.compile()
    return nc


_CACHE = {}


def _enable_ldw_opt():
    """Turn on walrus LDWEIGHTS double-buffering for this kernel's compile.

    concourse pins --enable-ldw-opt=false; without it every matmul
    serializes behind its weight load (~107ns per matmul at N=512).
    """
    if getattr(bass_utils, "_ldw_opt_patched", False):
        return
    orig = bass_utils.run_command

    def run_command_ldw(argv, **kwargs):
        argv = [
            "--enable-ldw-opt=true" if a == "--enable-ldw-opt=false" else a
            for a in argv
        ]
        return orig(argv, **kwargs)

    bass_utils.run_command = run_command_ldw
    bass_utils._ldw_opt_patched = True


def _get_program():
    # NOTE: _enable_ldw_opt is NOT called — walrus rejects every
    # bass-emitted InstLdweights under --enable-ldw-opt=true (verified by
    # bisection down to a single plain matmul on this compiler build).
    if "nc" not in _CACHE:
        _CACHE["nc"] = build_program()
    return _CACHE["nc"]


def _make_in_maps(inputs):
    import ml_dtypes

    x = np.asarray(inputs["x"], dtype=np.float32)
    Wq = np.asarray(inputs["Wq"], dtype=np.float32)
    Wk = np.asarray(inputs["Wk"], dtype=np.float32)
    Wv = np.asarray(inputs["Wv"], dtype=np.float32)
    wqk = np.ascontiguousarray(np.concatenate([Wq, Wk], axis=1)).astype(
        ml_dtypes.bfloat16
    )
    wv = np.ascontiguousarray(Wv).astype(ml_dtypes.bfloat16)
    iden = np.eye(128, dtype=np.float32)
    idenb = np.eye(128, dtype=ml_dtypes.bfloat16)
    mask = np.triu(np.ones((128, 128))).astype(ml_dtypes.bfloat16)
    in_maps = []
    for b in range(B):
        in_maps.append(
            {
                "xT": np.ascontiguousarray(
                    x[b].T.reshape(NK, 128, NJ, TB)
                    .transpose(1, 2, 0, 3)
                    .reshape(128, NJ * NK * TB)
                ).astype(ml_dtypes.bfloat16),
                "wqk": wqk,
                "wv": wv,
                "iden": iden,
                "idenb": idenb,
                "mask": mask,
            }
        )
    return in_maps


def kernel(x, Wk, Wq, Wv):
    nc = _get_program()
    in_maps = _make_in_maps({"x": x, "Wq": Wq, "Wk": Wk, "Wv": Wv})
    res = bass_utils.run_bass_kernel_spmd(nc, in_maps, core_ids=list(range(B)))
    return np.stack([res.results[b]["out"] for b in range(B)], axis=0)


# revision 18
# speedup vs baseline: 1.2009x; 1.2009x over previous
"""Single-head causal attention on 8 TRN2 NeuronCores (Bass/Tile).

Problem: x[B=8,T=4096,C=1024] @ {Wq,Wk,Wv}[C,HS=64] -> causal softmax
attention -> out[B,T,HS].

Sharding: data-parallel over batch — core b computes batch element b with
replicated projection weights (per the sharding hint).

Per-core dataflow (matmul operands bf16, fp32 PSUM accumulation):
  - x arrives transposed AND pre-cast to bf16 on the host (xT[C,T] bf16),
    so the DMA stream is half the bytes and no on-chip cast is needed.
  - [qT;kT] = [Wq|Wk]^T @ xT per 512-wide query block (PSUM-accumulated
    over 8 c-chunks); vT likewise; vT is PE-transposed back to natural
    v[s,64] with a ones-column appended so the PV matmul also produces
    softmax row-sums for free.
  - Scores are computed transposed (weiT[s,t] = kT^T @ qT) as row-packed
    pairs: two K=64 matmuls on disjoint PE row groups run concurrently
    (kT interleaved across partition halves, qT duplicated to the upper
    half), writing the two halves of one 2-bank PSUM tile.
  - exp runs on ScalarE straight out of PSUM (one 1024-wide ACT per pair)
    with the 1/sqrt(C) scale folded in; no running-max is needed (logits
    are small).  The pair loop is software-pipelined so the PE issues the
    next pair's score matmuls before the current pair's PV — ScalarE (the
    critical engine) never waits on the PE.
  - Causality: fully-masked 128-col strips are sliced off the score
    matmuls/PV and the 128x128 diagonal strips are masked via a DVE
    multiply with a 0/1 upper-triangular tile.
  - PV accumulates outT[65,512] over s-tiles in PSUM; finalize is a PE
    transpose + DVE reciprocal of the sums column + per-row scale.
  - Walrus LDWEIGHTS double-buffering is enabled (concourse pins it off)
    so matmuls don't serialize behind their weight loads.
"""

import numpy as np

import concourse.bacc as bacc
import concourse.bass as bass
import concourse.mybir as mybir
import concourse.tile as tile
from concourse import bass_utils

B, T, C, HS = 8, 4096, 1024, 64
TB = 512                 # query-block width (PSUM bank = 512 fp32)
NJ = T // TB             # 8 query blocks
NK = C // 128            # 8 contraction chunks
NS = T // 128            # 32 key tiles
SCALE = C ** -0.5

F32 = mybir.dt.float32
BF16 = mybir.dt.bfloat16
EXP = mybir.ActivationFunctionType.Exp


def build_program():
    nc = bacc.Bacc("TRN2", target_bir_lowering=False, debug=False)

    # x arrives host-permuted to [p, j, k, t] so each query-block chunk is
    # one contiguous 8KB run per partition -> 128 fat DMA descriptors
    # instead of 1024 thin ones (descriptor-dominated otherwise)
    xT = nc.dram_tensor("xT", [128, NJ * NK * TB], BF16, kind="ExternalInput")
    # all bf16 constants host-packed into one contiguous-per-partition
    # tensor: [wqk(8x128) | wv(8x64) | idenb(128) | mask(128)] = 1792 cols.
    # One DMA with 128 fat descriptors instead of ~2.5k tiny ones.
    consts = nc.dram_tensor("consts", [128, 1792], BF16, kind="ExternalInput")
    out = nc.dram_tensor("out", [T, HS], F32, kind="ExternalOutput")

    with tile.TileContext(nc) as tc:
        with (
            tc.tile_pool(name="const", bufs=1) as constp,
            tc.tile_pool(name="persist", bufs=1) as persist,
            tc.tile_pool(name="stage", bufs=2) as stg,
            tc.tile_pool(name="expp", bufs=4) as expp,
            tc.tile_pool(name="fin", bufs=8) as finp,
            tc.tile_pool(name="ps_pp", bufs=2, space=bass.MemorySpace.PSUM) as ps_pp,
            tc.tile_pool(name="ps_wei", bufs=2, space=bass.MemorySpace.PSUM) as ps_wei,
            tc.tile_pool(name="ps_out", bufs=2, space=bass.MemorySpace.PSUM) as ps_out,
        ):
            consts_sb = constp.tile([128, 1792], BF16)
            nc.scalar.dma_start(consts_sb[:], consts[:])
            wqk_sb = consts_sb[:, 0:1024].rearrange("p (k m) -> p k m", m=128)
            wv_sb = consts_sb[:, 1024:1536].rearrange("p (k m) -> p k m", m=HS)
            idenb_sb = consts_sb[:, 1536:1664]
            mask_sb = consts_sb[:, 1664:1792]
            # tiny dummy exp: pulls ACT_TABLE_LOAD (~2.7us) into the DMA head
            warm = finp.tile([1, 1], F32, tag="warm", bufs=1)
            nc.scalar.activation(warm[:], consts_sb[0:1, 0:1], EXP, scale=SCALE)

            # whole xT resident in SBUF (bf16, 64KB/partition), streamed in
            # per-block chunks so compute can start after the first chunk;
            # layout [p, j, k, t] keeps each chunk contiguous per partition
            xt = persist.tile([128, NJ, NK, TB], BF16)
            # keys, transposed + interleaved: pair p holds kT of s-tile 2p on
            # partitions 0-63 and of s-tile 2p+1 on partitions 64-127
            kTI = persist.tile([128, (NS // 2) * 128], BF16)
            # values + ones col, padded to 80 so each s-tile's row offset is
            # 32B-aligned (required by the xbar DMA-transpose destination)
            v_all = persist.tile([128, NS, 80], BF16)
            nc.vector.memset(v_all[:, :, HS : HS + 1], 1.0)

            # x stream: first two chunks ride the sync HWDGE ring (fast
            # first-byte — the head of the kernel waits on them), the rest
            # ride SWDGE (gpsimd) so they never contend with the small
            # latency-critical SBUF->SBUF transfers on the sync ring.
            for jj in range(NJ):
                eng = nc.scalar if jj < 3 else nc.gpsimd
                eng.dma_start(
                    xt[:, jj, :, :],
                    xT[:, jj * NK * TB : (jj + 1) * NK * TB].rearrange(
                        "p (k t) -> p k t", t=TB
                    ),
                )

            # Projections run ONE BLOCK AHEAD of the pair loop so every
            # wei/PV dependency (qkt, qt2, kTI, v_all) is produced a full
            # block early — the small sync-ring DMAs then have a whole
            # block of slack and never stall the PE.
            def proj_qk(j):
                t0 = j * TB
                # [qT;kT] projection: rows 0-63 = qT, rows 64-127 = kT
                qk_ps = ps_pp.tile([128, TB], F32, tag="pp", name="qk_ps")
                for k in range(NK):
                    nc.tensor.matmul(
                        qk_ps[:], wqk_sb[:, k, :], xt[:, j, k, :],
                        start=(k == 0), stop=(k == NK - 1),
                    )
                qkt = stg.tile([128, TB], BF16, tag="qkt", name="qkt")
                nc.vector.tensor_copy(qkt[:], qk_ps[:])
                # duplicate qT onto partitions 64-127 (row-packed QK rhs)
                qt2 = stg.tile([128, TB], BF16, tag="qt2", name="qt2")
                nc.sync.dma_start(qt2[64:128, :], qkt[0:64, :])
                # interleave this block's 4 kT tiles into the pair layout:
                # even tiles -> partitions 0-63, odd tiles -> 64-127
                kt_src = qkt[64:128, :].rearrange(
                    "p (a e b) -> p a e b", e=2, b=128
                )
                kt_dst = kTI[:, 256 * j : 256 * (j + 1)].rearrange(
                    "p (a b) -> p a b", b=128
                )
                nc.sync.dma_start(kt_dst[0:64, :, :], kt_src[:, :, 0, :])
                nc.sync.dma_start(kt_dst[64:128, :, :], kt_src[:, :, 1, :])
                return qkt, qt2

            def proj_v(j):
                t0 = j * TB
                # v projection, row-packed across partition halves, then the
                # halves are folded and DMA-transposed to natural v[s,64]
                vt_ps = ps_pp.tile([128, TB], F32, tag="pp", name="vt_ps")
                for k in range(NK):
                    lo = HS * (k % 2)
                    nc.tensor.matmul(
                        vt_ps[lo : lo + HS, :], wv_sb[:, k, :],
                        xt[:, j, k, :],
                        start=(k <= 1), stop=(k >= NK - 2),
                        skip_group_check=True,
                    )
                vt_hi = stg.tile([128, TB], F32, tag="vt_hi", name="vt_hi")
                nc.vector.tensor_copy(vt_hi[64:128, :], vt_ps[64:128, :])
                vt_lo = stg.tile([HS, TB], F32, tag="vt_lo", name="vt_lo")
                nc.sync.dma_start(vt_lo[:], vt_hi[64:128, :])
                vt_sb = stg.tile([HS, TB], BF16, tag="vt_sb", name="vt_sb")
                nc.vector.tensor_add(vt_sb[:], vt_ps[0:HS, :], vt_lo[:])
                for rr in range(TB // 128):
                    nc.sync.dma_start_transpose(
                        v_all[:, 4 * j + rr, 0:HS],
                        vt_sb[:, rr * 128 : (rr + 1) * 128],
                    )

            def issue_wei(j, qkt, qt2, p):
                iA, iB = 2 * p, 2 * p + 1
                rA, rB = iA - 4 * j, iB - 4 * j
                c0A = 128 * rA if rA > 0 else 0
                c0B = 128 * rB if rB > 0 else 0
                wei = ps_wei.tile([128, 2 * TB], F32, tag="wei", name="wei")
                nc.tensor.matmul(
                    wei[:, c0A:TB],
                    kTI[0:64, 128 * p : 128 * (p + 1)],
                    qkt[0:HS, c0A:TB],
                    start=True, stop=True,
                )
                nc.tensor.matmul(
                    wei[:, TB + c0B : 2 * TB],
                    kTI[64:128, 128 * p : 128 * (p + 1)],
                    qt2[64:128, c0B:TB],
                    start=True, stop=True,
                )
                return wei, c0A, c0B, rA, rB

            cur = proj_qk(0)
            proj_v(0)
            # prime the wei pipeline (2 tiles = full psum ring)
            pend = [issue_wei(0, *cur, 0), issue_wei(0, *cur, 1)]

            for j in range(NJ):
                t0 = j * TB
                qkt, qt2 = cur
                n_pairs = 2 * j + 2

                outT_ps = ps_out.tile([HS + 1, TB], F32, tag="outT")
                for p in range(n_pairs):
                    wei, c0A, c0B, rA, rB = pend.pop(0)
                    iA, iB = 2 * p, 2 * p + 1
                    ex = expp.tile([128, 2 * TB], BF16, tag="exp")
                    # one ACT covers both halves; the dead [TB, TB+c0B) gap
                    # holds bounded stale scores and is never read by PV
                    nc.scalar.activation(
                        ex[:, c0A : 2 * TB], wei[:, c0A : 2 * TB], EXP,
                        scale=SCALE,
                    )
                    if rA >= 0:
                        nc.vector.tensor_mul(
                            ex[:, c0A : c0A + 128], ex[:, c0A : c0A + 128],
                            mask_sb[:],
                        )
                    if rB >= 0:
                        nc.vector.tensor_mul(
                            ex[:, TB + c0B : TB + c0B + 128],
                            ex[:, TB + c0B : TB + c0B + 128],
                            mask_sb[:],
                        )
                    if p + 2 < n_pairs:
                        pend.append(issue_wei(j, qkt, qt2, p + 2))
                    # next block's projections slot into the PE stream here
                    # (before PV, which can wait) so their results are ready
                    # a full block early
                    if p == 0 and j + 1 < NJ:
                        nxt = proj_qk(j + 1)
                    if p == 1 and j + 1 < NJ:
                        proj_v(j + 1)
                    nc.tensor.matmul(
                        outT_ps[:, c0A:TB],
                        v_all[:, iA, 0 : HS + 1],
                        ex[:, c0A:TB],
                        start=(p == 0), stop=False,
                        skip_group_check=True,
                    )
                    nc.tensor.matmul(
                        outT_ps[:, c0B:TB],
                        v_all[:, iB, 0 : HS + 1],
                        ex[:, TB + c0B : 2 * TB],
                        start=False, stop=(p == n_pairs - 1),
                        skip_group_check=True,
                    )

                # prime next block's first two score pairs BEFORE finalize,
                # so ScalarE's next ACT never waits behind the fin chain
                if j + 1 < NJ:
                    cur = nxt
                    pend = [
                        issue_wei(j + 1, *cur, 0),
                        issue_wei(j + 1, *cur, 1),
                    ]

                # finalize in bf16 (fp32 PE transpose runs in slow fp32
                # mode; bf16 rounding of outT adds ~0.3% well within budget)
                outT_sb = stg.tile([HS + 1, TB], BF16, tag="outT_sb")
                nc.vector.tensor_copy(outT_sb[:], outT_ps[:])
                o = finp.tile([128, TB // 128, HS], F32, tag="o", bufs=2)
                for rr in range(TB // 128):
                    fin_ps = ps_pp.tile([128, HS + 1], BF16, tag="pp")
                    nc.tensor.transpose(
                        fin_ps[:], outT_sb[:, rr * 128 : (rr + 1) * 128],
                        idenb_sb[: HS + 1, : HS + 1],
                    )
                    rec = finp.tile([128, 1], F32, tag="rec")
                    nc.vector.reciprocal(rec[:], fin_ps[:, HS : HS + 1])
                    nc.vector.tensor_scalar_mul(
                        o[:, rr, :], fin_ps[:, 0:HS], rec[:]
                    )
                # one coalesced store per block, on SWDGE (leaf transfer —
                # nothing downstream waits on it)
                nc.gpsimd.dma_start(
                    out[t0 : t0 + TB, :].rearrange("(a p) h -> p a h", p=128),
                    o[:],
                )

    nc.compile()
    return nc


_CACHE = {}


def _enable_ldw_opt():
    """Turn on walrus LDWEIGHTS double-buffering for this kernel's compile.

    concourse pins --enable-ldw-opt=false; without it every matmul
    serializes behind its weight load (~107ns per matmul at N=512).
    """
    if getattr(bass_utils, "_ldw_opt_patched", False):
        return
    orig = bass_utils.run_command

    def run_command_ldw(argv, **kwargs):
        argv = [
            "--enable-ldw-opt=true" if a == "--enable-ldw-opt=false" else a
            for a in argv
        ]
        return orig(argv, **kwargs)

    bass_utils.run_command = run_command_ldw
    bass_utils._ldw_opt_patched = True


def _get_program():
    # NOTE: _enable_ldw_opt is NOT called — walrus rejects every
    # bass-emitted InstLdweights under --enable-ldw-opt=true (verified by
    # bisection down to a single plain matmul on this compiler build).
    if "nc" not in _CACHE:
        _CACHE["nc"] = build_program()
    return _CACHE["nc"]


def _make_in_maps(inputs):
    import ml_dtypes

    x = np.asarray(inputs["x"], dtype=np.float32)
    Wq = np.asarray(inputs["Wq"], dtype=np.float32)
    Wk = np.asarray(inputs["Wk"], dtype=np.float32)
    Wv = np.asarray(inputs["Wv"], dtype=np.float32)
    wqk = np.concatenate([Wq, Wk], axis=1)  # [C, 128]
    consts = np.concatenate(
        [
            wqk.reshape(NK, 128, 128).transpose(1, 0, 2).reshape(128, 1024),
            Wv.reshape(NK, 128, HS).transpose(1, 0, 2).reshape(128, 512),
            np.eye(128, dtype=np.float32),
            np.triu(np.ones((128, 128), dtype=np.float32)),
        ],
        axis=1,
    ).astype(ml_dtypes.bfloat16)
    consts = np.ascontiguousarray(consts)
    in_maps = []
    for b in range(B):
        in_maps.append(
            {
                "xT": np.ascontiguousarray(
                    x[b].T.reshape(NK, 128, NJ, TB)
                    .transpose(1, 2, 0, 3)
                    .reshape(128, NJ * NK * TB)
                ).astype(ml_dtypes.bfloat16),
                "consts": consts,
            }
        )
    return in_maps


def kernel(x, Wk, Wq, Wv):
    nc = _get_program()
    in_maps = _make_in_maps({"x": x, "Wq": Wq, "Wk": Wk, "Wv": Wv})
    res = bass_utils.run_bass_kernel_spmd(nc, in_maps, core_ids=list(range(B)))
    return np.stack([res.results[b]["out"] for b in range(B)], axis=0)
